# revision 1
# baseline (speedup 1.0000x reference)
"""TRN2 Bass kernel: 3-layer GIN (sum-agg) + MLP + BatchNorm + graph sum-pooling + linear.

Full inputs in, full output out. Internally: 8-way data parallel over nodes
(12500 contiguous nodes/core), SPMD NEFF via run_bass_kernel_spmd.

Per layer l on each core (feature-major f32 working set [64, 12544]):
  l==0: x = (1+eps0)*emb[nid]+agg factors through the 1600-wide vocabulary:
        x^T_blk = sum_vc emb16_chunk^T @ (counts + (1+eps)*onehot(nid)) — no gathers.
  l>0 : agg^T_blk accumulates in PSUM over indirect-DMA row gathers (128 rows/call,
        fp16 h table, two 23-bit windows via element_offset) matmul'ed against
        one-hot dst matrices built on DVE.
  Then 3x(Linear+ReLU) on PE/ACT, BN stats cross-core via AllReduce, per-graph
  pooling via one-hot matmul, h writeback (PE transpose -> fp16 rows) + AllGather.
Final: pooled^T_l @ W_out slices accumulate -> [128,100], indirect-scatter into
[513,100] by graph window, AllReduce over cores, + b_out.
"""
import math
import sys
import types

import numpy as np

HID = 64
P = 128
NCORES = 8
VOCAB = 3100
ID_OFFSET = 1500
NUM_CLASSES = 100
BN_EPS = 1e-5

CFG = dict(
    n_nodes=100_000,
    n_graphs=512,
    win_rows=65_536,   # fp16 row reach of 23-bit dynamic offset
    blk=512,           # dst nodes per PSUM block
    oh_batch=4,        # one-hot tiles built per DVE op
)

_PROFILE = False
_LAST_EXEC_NS = None


def _install_profile_hook():
    try:
        import antenv
        from trn_agent_boot.trn_boot import _ntff_profile_via_ctypes
    except Exception:
        return False
    if "antenv.axon_hooks" in sys.modules:
        return True
    hooks = types.ModuleType("antenv.axon_hooks")
    hooks._hook = _ntff_profile_via_ctypes("/opt/axon/libaxon_pjrt.so")
    hooks.set_axon_ntff_profile_hook = lambda h: setattr(hooks, "_hook", h)
    hooks.get_axon_ntff_profile_hook = lambda: hooks._hook
    sys.modules["antenv.axon_hooks"] = hooks
    antenv.axon_hooks = hooks
    return True


def _derived(cfg):
    n = cfg["n_nodes"]
    loc = n // NCORES
    lpad = ((loc + P - 1) // P) * P
    nblk = (lpad + cfg["blk"] - 1) // cfg["blk"]
    ntn = lpad // P
    vpad = ((VOCAB - ID_OFFSET + P - 1) // P) * P   # 1664
    nvc = vpad // P
    return loc, lpad, nblk, ntn, vpad, nvc


def _prep(cfg, node_ids, edge_src, edge_dst, graph_ids, Ws, bs, bn_gamma,
          bn_beta, eps, W_out, b_out, emb):
    """Index-only host preprocessing -> per-core input dicts + compile caps."""
    loc, lpad, nblk, ntn, vpad, nvc = _derived(cfg)
    n, wr, blk = cfg["n_nodes"], cfg["win_rows"], cfg["blk"]
    node_ids = np.asarray(node_ids, np.int64)
    edge_src = np.asarray(edge_src, np.int64)
    edge_dst = np.asarray(edge_dst, np.int64)
    graph_ids = np.asarray(graph_ids, np.int64)

    per_core = []
    # pass 1: group edges, find caps
    grp = []   # per core: dict (b, w) -> (src_idx_rel, dst_in_blk)
    for c in range(NCORES):
        base = c * loc
        m = (edge_dst >= base) & (edge_dst < base + loc)
        src = edge_src[m]
        dl = edge_dst[m] - base
        b = dl // blk
        w = (src >= wr).astype(np.int64)
        g = {}
        order = np.lexsort((dl, w, b))
        src, dl, b, w = src[order], dl[order], b[order], w[order]
        key = b * 2 + w
        cuts = np.searchsorted(key, np.arange(nblk * 2 + 1))
        for bb in range(nblk):
            for ww in range(2):
                k = bb * 2 + ww
                s, e = cuts[k], cuts[k + 1]
                g[(bb, ww)] = (src[s:e] - ww * wr, dl[s:e] - bb * blk)
        grp.append(g)

    cap = [0, 0]
    for c in range(NCORES):
        for (bb, ww), (s, _) in grp[c].items():
            cap[ww] = max(cap[ww], len(s))
    tiles_w = [max(1, (cap[w] + P - 1) // P) for w in range(2)]
    tot_t = tiles_w[0] + tiles_w[1]
    ncalls = nblk * tot_t

    # counts matrix + slot arrays per core
    for c in range(NCORES):
        base = c * loc
        eidx = np.zeros((P, ncalls), np.int32)
        edst = np.full((P, ncalls), -1.0, np.float32)
        for bb in range(nblk):
            for ww in range(2):
                s, d = grp[c][(bb, ww)]
                cal0 = bb * tot_t + (0 if ww == 0 else tiles_w[0])
                nt_ = tiles_w[ww]
                si = np.zeros(nt_ * P, np.int32)
                di = np.full(nt_ * P, -1.0, np.float32)
                si[:len(s)] = s
                di[:len(s)] = d.astype(np.float32)
                eidx[:, cal0:cal0 + nt_] = si.reshape(nt_, P).T
                edst[:, cal0:cal0 + nt_] = di.reshape(nt_, P).T

        # vocab count matrix cntT [vpad, lpad] fp16
        m = (edge_dst >= base) & (edge_dst < base + loc)
        src = edge_src[m]
        dl = edge_dst[m] - base
        v = node_ids[src]          # 0..VOCAB-ID_OFFSET-1 range by construction
        cnt = np.bincount(v * lpad + dl, minlength=vpad * lpad)
        cntT = cnt.reshape(vpad, lpad).astype(np.float16)

        nidrep = np.full((P, lpad), -1.0, np.float16)
        nidrep[:, :loc] = np.tile(node_ids[base:base + loc].astype(np.float16),
                                  (P, 1))
        gl = np.full((P, ntn), -1.0, np.float32)
        g_base = int(graph_ids[base])
        g_span = int(graph_ids[base + loc - 1]) - g_base
        assert g_span < P, f"graph window {g_span} >= {P}"
        glv = (graph_ids[base:base + loc] - g_base).astype(np.float32)
        gl_full = np.full(lpad, -1.0, np.float32)
        gl_full[:loc] = glv
        gl[:, :] = gl_full.reshape(ntn, P).T
        growidx = np.minimum(g_base + np.arange(P), cfg["n_graphs"]).astype(
            np.int32)[:, None]

        wpack = np.zeros((HID, 9 * HID), np.float32)
        for l in range(3):
            for mm in range(3):
                wpack[:, (3 * l + mm) * HID:(3 * l + mm + 1) * HID] = Ws[l, mm]
        woutp = np.zeros((HID, 3 * NUM_CLASSES), np.float32)
        for l in range(3):
            woutp[:, l * NUM_CLASSES:(l + 1) * NUM_CLASSES] = \
                W_out[l * HID:(l + 1) * HID]

        per_core.append(dict(
            emb=np.asarray(emb, np.float32),
            cntT=cntT,
            nidrep=nidrep,
            eidx=eidx,
            edst=edst,
            gl=gl,
            growidx=growidx,
            iotaV=(np.arange(P, dtype=np.float32)[:, None]
                   + P * np.arange(nvc, dtype=np.float32)[None, :]),
            iota512=np.tile(np.arange(blk, dtype=np.float32)[None, :], (P, 1)),
            iota128=np.tile(np.arange(P, dtype=np.float32)[None, :], (P, 1)),
            idn64=np.eye(HID, dtype=np.float32),
            wpack=wpack,
            bsT=np.asarray(bs, np.float32).reshape(9, HID).T.copy(),
            gammaT=np.asarray(bn_gamma, np.float32).T.copy(),
            betaT=np.asarray(bn_beta, np.float32).T.copy(),
            eps_rep=np.tile(np.asarray(eps, np.float32)[None, :], (P, 1)),
            woutp=woutp,
            boutr=np.tile(np.asarray(b_out, np.float32)[None, :], (P, 1)),
        ))
    return per_core, tiles_w, ncalls


def _build(cfg, tiles_w, ncalls):
    import concourse.bacc as bacc
    import concourse.bass as bass
    import concourse.mybir as mybir
    import concourse.tile as tile

    loc, lpad, nblk, ntn, vpad, nvc = _derived(cfg)
    n, g, wr, blk = (cfg["n_nodes"], cfg["n_graphs"], cfg["win_rows"],
                     cfg["blk"])
    OHB = cfg["oh_batch"]
    f32, f16, i32 = mybir.dt.float32, mybir.dt.float16, mybir.dt.int32
    AL, AF = mybir.AluOpType, mybir.ActivationFunctionType
    tot_t = tiles_w[0] + tiles_w[1]
    emb_rows = vpad + P  # emb16 tensor rows: need ID_OFFSET..ID_OFFSET+vpad; keep simple pad

    nc = bacc.Bacc()
    D = {}
    def di(name, shape, dt):
        D[name] = nc.dram_tensor(name, shape, dt, kind="ExternalInput")
        return D[name]

    emb = di("emb", [VOCAB, HID], f32)
    cntT = di("cntT", [vpad, lpad], f16)
    nidrep = di("nidrep", [P, lpad], f16)
    eidx = di("eidx", [P, ncalls], i32)
    edst = di("edst", [P, ncalls], f32)
    gl = di("gl", [P, ntn], f32)
    growidx = di("growidx", [P, 1], i32)
    iotaV = di("iotaV", [P, nvc], f32)
    iota512 = di("iota512", [P, blk], f32)
    iota128 = di("iota128", [P, P], f32)
    idn64 = di("idn64", [HID, HID], f32)
    wpack = di("wpack", [HID, 9 * HID], f32)
    bsT = di("bsT", [HID, 9], f32)
    gammaT = di("gammaT", [HID, 3], f32)
    betaT = di("betaT", [HID, 3], f32)
    eps_rep = di("eps_rep", [P, 3], f32)
    woutp = di("woutp", [HID, 3 * NUM_CLASSES], f32)
    boutr = di("boutr", [P, NUM_CLASSES], f32)

    emb16 = nc.dram_tensor("emb16", [ID_OFFSET + vpad, HID], f16)
    hdev = [nc.dram_tensor(f"hdev{i}", [loc, HID], f16) for i in range(2)]
    hfull = [nc.dram_tensor(f"hfull{i}", [n, HID], f16, addr_space="Shared")
             for i in range(2)]
    statsin = [nc.dram_tensor(f"statsin{i}", [HID, 2], f32) for i in range(3)]
    statsout = [nc.dram_tensor(f"statsout{i}", [HID, 2], f32,
                               addr_space="Shared") for i in range(3)]
    obig = nc.dram_tensor("obig", [g + 1, NUM_CLASSES], f32)
    obig_red = nc.dram_tensor("obig_red", [g + 1, NUM_CLASSES], f32,
                              addr_space="Shared")
    out = nc.dram_tensor("out", [g, NUM_CLASSES], f32, kind="ExternalOutput")

    RG = [list(range(NCORES))]

    with tile.TileContext(nc) as tc:
        with (
            tc.tile_pool(name="c1", bufs=1) as c1,
            tc.tile_pool(name="cnt", bufs=3) as cntp,
            tc.tile_pool(name="rhs", bufs=3) as rhsp,
            tc.tile_pool(name="hs", bufs=64) as hsp,
            tc.tile_pool(name="oh", bufs=3) as ohp,
            tc.tile_pool(name="sb", bufs=4) as sbp,
            tc.tile_pool(name="rows", bufs=3) as rowsp,
            tc.tile_pool(name="psx", bufs=2, space="PSUM") as psx,
            tc.tile_pool(name="psm", bufs=2, space="PSUM") as psm,
            tc.tile_pool(name="pst", bufs=2, space="PSUM") as pst,
            tc.tile_pool(name="psp", bufs=1, space="PSUM") as psp,
        ):
            # ---- constants to SBUF ----
            def load(tname, dram, shape, dt):
                t = c1.tile(shape, dt, tag=tname)
                nc.sync.dma_start(out=t[:], in_=dram[:])
                return t
            eidx_sb = load("eidx", eidx, [P, ncalls], i32)
            edst_sb = load("edst", edst, [P, ncalls], f32)
            nid_sb = load("nidrep", nidrep, [P, lpad], f16)
            gl_sb = load("gl", gl, [P, ntn], f32)
            grow_sb = load("growidx", growidx, [P, 1], i32)
            iV_sb = load("iotaV", iotaV, [P, nvc], f32)
            i512_sb = load("iota512", iota512, [P, blk], f32)
            i128_sb = load("iota128", iota128, [P, P], f32)
            idn_sb = load("idn64", idn64, [HID, HID], f32)
            w_sb = load("wpack", wpack, [HID, 9 * HID], f32)
            b_sb = load("bsT", bsT, [HID, 9], f32)
            gam_sb = load("gammaT", gammaT, [HID, 3], f32)
            bet_sb = load("betaT", betaT, [HID, 3], f32)
            eps_sb = load("eps_rep", eps_rep, [P, 3], f32)
            wo_sb = load("woutp", woutp, [HID, 3 * NUM_CLASSES], f32)
            bo_sb = load("boutr", boutr, [P, NUM_CLASSES], f32)

            e1p32 = c1.tile([P, 3], f32)
            nc.vector.tensor_scalar(out=e1p32[:], in0=eps_sb[:], scalar1=1.0,
                                    scalar2=None, op0=AL.add)

            # ---- emb cast f32 -> f16 (rows ID_OFFSET.. only are used) ----
            zt16 = c1.tile([P, HID], f16)
            nc.vector.memset(zt16[:], 0.0)
            nrt = (VOCAB - ID_OFFSET + P - 1) // P   # chunks starting at ID_OFFSET
            for t in range(nrt):
                r0 = ID_OFFSET + t * P
                r1 = min(r0 + P, VOCAB)
                et = rowsp.tile([P, HID], f32, tag="embcast")
                nc.sync.dma_start(out=et[:r1 - r0, :], in_=emb[r0:r1, :])
                et16 = rowsp.tile([P, HID], f16, tag="embcast16")
                nc.vector.tensor_copy(out=et16[:r1 - r0, :], in_=et[:r1 - r0, :])
                if r1 - r0 < P:
                    nc.vector.tensor_copy(out=et16[r1 - r0:, :],
                                          in_=zt16[:P - (r1 - r0), :])
                nc.sync.dma_start(out=emb16[r0:r0 + P, :], in_=et16[:])
            # chunks fully beyond VOCAB (zero counts) -> zero rows
            for t in range(nrt, nvc):
                r0 = ID_OFFSET + t * P
                nc.sync.dma_start(out=emb16[r0:r0 + P, :], in_=zt16[:])

            embc = c1.tile([P, nvc * HID], f16)
            for vc in range(nvc):
                nc.sync.dma_start(
                    out=embc[:, vc * HID:(vc + 1) * HID],
                    in_=emb16[ID_OFFSET + vc * P:ID_OFFSET + (vc + 1) * P, :])

            hTown = c1.tile([HID, lpad], f32)
            x3f = c1.tile([HID, lpad], f32)
            pooledT = c1.tile([HID, 3 * P], f32)

            # ================= layers =================
            for l in range(3):
                for b in range(nblk):
                    bs0 = b * blk
                    bw = min(blk, lpad - bs0)
                    if l == 0:
                        ps_x = psx.tile([HID, blk], f32, space="PSUM", tag="psx")
                        for vc in range(nvc):
                            ct = cntp.tile([P, blk], f16)
                            nc.sync.dma_start(
                                out=ct[:, :bw],
                                in_=cntT[vc * P:(vc + 1) * P, bs0:bs0 + bw])
                            r2 = rhsp.tile([P, blk], f16)
                            nc.vector.tensor_scalar(
                                out=r2[:, :bw], in0=nid_sb[:, bs0:bs0 + bw],
                                scalar1=iV_sb[:, vc:vc + 1],
                                scalar2=e1p32[:, 0:1],
                                op0=AL.is_equal, op1=AL.mult)
                            nc.vector.tensor_tensor(
                                out=r2[:, :bw], in0=r2[:, :bw], in1=ct[:, :bw],
                                op=AL.add)
                            nc.tensor.matmul(
                                ps_x[:, :bw],
                                lhsT=embc[:, vc * HID:(vc + 1) * HID],
                                rhs=r2[:, :bw],
                                start=(vc == 0), stop=(vc == nvc - 1))
                        xT = sbp.tile([HID, blk], f32, tag="xT")
                        nc.vector.tensor_copy(out=xT[:, :bw], in_=ps_x[:, :bw])
                        xsrc = xT
                    else:
                        hprev = hfull[l - 1]
                        ps_a = psx.tile([HID, blk], f32, space="PSUM", tag="psx")
                        cal0 = b * tot_t
                        for t0 in range(0, tot_t, OHB):
                            nb = min(OHB, tot_t - t0)
                            oh = ohp.tile([P, OHB * blk], f16)
                            a0 = edst_sb[:, cal0 + t0:cal0 + t0 + nb]
                            in0 = bass.AP(a0.tensor, a0.offset,
                                          [a0.ap[0], [1, nb], [0, blk]])
                            a1 = i512_sb[:]
                            in1 = bass.AP(a1.tensor, a1.offset,
                                          [a1.ap[0], [0, nb], [1, blk]])
                            nc.vector.tensor_tensor(
                                out=oh[:, :nb * blk], in0=in0, in1=in1,
                                op=AL.is_equal)
                            for tt in range(nb):
                                t = t0 + tt
                                cal = cal0 + t
                                w = 0 if t < tiles_w[0] else 1
                                hs = hsp.tile([P, HID], f16)
                                nc.gpsimd.indirect_dma_start(
                                    out=hs[:], out_offset=None, in_=hprev[:],
                                    in_offset=bass.IndirectOffsetOnAxis(
                                        ap=eidx_sb[:, cal:cal + 1], axis=0),
                                    element_offset=(wr * HID if w == 1 else 0))
                                nc.tensor.matmul(
                                    ps_a[:, :bw], lhsT=hs[:],
                                    rhs=oh[:, tt * blk:tt * blk + bw],
                                    start=(t == 0), stop=(t == tot_t - 1))
                        xT = sbp.tile([HID, blk], f32, tag="xT")
                        nc.vector.tensor_scalar(
                            out=xT[:, :bw], in0=hTown[:, bs0:bs0 + bw],
                            scalar1=e1p32[:HID, l:l + 1], scalar2=None,
                            op0=AL.mult)
                        nc.vector.tensor_tensor(
                            out=xT[:, :bw], in0=xT[:, :bw], in1=ps_a[:, :bw],
                            op=AL.add)
                        xsrc = xT
                    cur = xsrc
                    for m in range(3):
                        ps_m = psm.tile([HID, blk], f32, space="PSUM", tag="psm")
                        nc.tensor.matmul(
                            ps_m[:, :bw],
                            lhsT=w_sb[:, (3 * l + m) * HID:(3 * l + m + 1) * HID],
                            rhs=cur[:, :bw],
                            start=True, stop=True)
                        if m == 2:
                            dst_t = x3f[:, bs0:bs0 + bw]
                        else:
                            nxt = sbp.tile([HID, blk], f32, tag="mlp")
                            dst_t = nxt[:, :bw]
                        nc.scalar.activation(
                            out=dst_t, in_=ps_m[:, :bw], func=AF.Relu,
                            bias=b_sb[:, 3 * l + m:3 * l + m + 1])
                        cur = dst_t if m == 2 else nxt

                # ---- BN stats + AllReduce ----
                st = sbp.tile([HID, 2], f32, tag="st")
                nc.vector.tensor_reduce(out=st[:, 0:1], in_=x3f[:, :loc],
                                        axis=mybir.AxisListType.X, op=AL.add)
                nc.scalar.activation(out=hTown[:, :loc], in_=x3f[:, :loc],
                                     func=AF.Square, accum_out=st[:, 1:2])
                nc.sync.dma_start(out=statsin[l][:], in_=st[:])
                nc.gpsimd.collective_compute(
                    "AllReduce", AL.add, replica_groups=RG,
                    ins=[statsin[l][:]], outs=[statsout[l][:]])
                rd = sbp.tile([HID, 2], f32, tag="rd")
                nc.sync.dma_start(out=rd[:], in_=statsout[l][:])
                mv = sbp.tile([HID, 1], f32, tag="mv")
                nc.vector.tensor_scalar(out=mv[:], in0=rd[:, 0:1],
                                        scalar1=1.0 / n, scalar2=None,
                                        op0=AL.mult)
                vr = sbp.tile([HID, 1], f32, tag="vr")
                nc.vector.tensor_scalar(out=vr[:], in0=rd[:, 1:2],
                                        scalar1=1.0 / n, scalar2=None,
                                        op0=AL.mult)
                m2 = sbp.tile([HID, 1], f32, tag="m2")
                nc.vector.tensor_tensor(out=m2[:], in0=mv[:], in1=mv[:],
                                        op=AL.mult)
                nc.vector.tensor_tensor(out=vr[:], in0=vr[:], in1=m2[:],
                                        op=AL.subtract)
                nc.vector.tensor_scalar(out=vr[:], in0=vr[:], scalar1=BN_EPS,
                                        scalar2=None, op0=AL.add)
                sq = sbp.tile([HID, 1], f32, tag="sq")
                nc.scalar.activation(out=sq[:], in_=vr[:], func=AF.Sqrt)
                inv = sbp.tile([HID, 1], f32, tag="inv")
                nc.vector.reciprocal(out=inv[:], in_=sq[:])
                scl = sbp.tile([HID, 1], f32, tag="scl")
                nc.vector.tensor_tensor(out=scl[:], in0=inv[:],
                                        in1=gam_sb[:, l:l + 1], op=AL.mult)
                shf = sbp.tile([HID, 1], f32, tag="shf")
                nc.vector.tensor_tensor(out=shf[:], in0=mv[:], in1=scl[:],
                                        op=AL.mult)
                nc.vector.tensor_tensor(out=shf[:], in0=bet_sb[:, l:l + 1],
                                        in1=shf[:], op=AL.subtract)
                # h_{l+1} = x3*scl + shf  (overwrites hTown)
                nc.vector.tensor_scalar(out=hTown[:], in0=x3f[:],
                                        scalar1=scl[:], scalar2=shf[:],
                                        op0=AL.mult, op1=AL.add)

                # ---- rows pass: transpose, pool, writeback ----
                ps_p = psp.tile([HID, P], f32, space="PSUM", tag="psp")
                for nt in range(ntn):
                    ps_t = pst.tile([P, HID], f32, space="PSUM", tag="pst")
                    nc.tensor.transpose(ps_t[:], hTown[:, nt * P:(nt + 1) * P],
                                        idn_sb[:])
                    r16 = rowsp.tile([P, HID], f16, tag="r16")
                    nc.scalar.activation(out=r16[:], in_=ps_t[:], func=AF.Copy)
                    if l < 2:
                        nr = min(P, loc - nt * P)
                        if nr > 0:
                            nc.sync.dma_start(out=hdev[l][nt * P:nt * P + nr, :],
                                              in_=r16[:nr, :])
                    gh = rowsp.tile([P, P], f16, tag="gh")
                    nc.vector.tensor_scalar(out=gh[:], in0=i128_sb[:],
                                            scalar1=gl_sb[:, nt:nt + 1],
                                            scalar2=None, op0=AL.is_equal)
                    nc.tensor.matmul(ps_p[:], lhsT=r16[:], rhs=gh[:],
                                     start=(nt == 0), stop=(nt == ntn - 1))
                nc.vector.tensor_copy(out=pooledT[:, l * P:(l + 1) * P],
                                      in_=ps_p[:])
                if l < 2:
                    nc.gpsimd.collective_compute(
                        "AllGather", AL.bypass, replica_groups=RG,
                        ins=[hdev[l][:]], outs=[hfull[l][:]])

            # ================= final linear =================
            ps_o = psp.tile([P, NUM_CLASSES], f32, space="PSUM", tag="psp")
            for l in range(3):
                nc.tensor.matmul(
                    ps_o[:], lhsT=pooledT[:, l * P:(l + 1) * P],
                    rhs=wo_sb[:, l * NUM_CLASSES:(l + 1) * NUM_CLASSES],
                    start=(l == 0), stop=(l == 2))
            oloc = sbp.tile([P, NUM_CLASSES], f32, tag="oloc")
            nc.vector.tensor_copy(out=oloc[:], in_=ps_o[:])
            zt = sbp.tile([P, NUM_CLASSES], f32, tag="zt")
            nc.vector.memset(zt[:], 0.0)
            nzt = (g + 1 + P - 1) // P
            for r in range(nzt):
                r0 = r * P
                r1 = min(r0 + P, g + 1)
                nc.sync.dma_start(out=obig[r0:r1, :], in_=zt[:r1 - r0, :])
            nc.gpsimd.indirect_dma_start(
                out=obig[:], out_offset=bass.IndirectOffsetOnAxis(
                    ap=grow_sb[:, 0:1], axis=0),
                in_=oloc[:], in_offset=None)
            nc.gpsimd.collective_compute(
                "AllReduce", AL.add, replica_groups=RG,
                ins=[obig[:]], outs=[obig_red[:]])
            for r in range((g + P - 1) // P):
                r0 = r * P
                r1 = min(r0 + P, g)
                ot = sbp.tile([P, NUM_CLASSES], f32, tag="ot")
                nc.sync.dma_start(out=ot[:r1 - r0, :], in_=obig_red[r0:r1, :])
                nc.vector.tensor_tensor(out=ot[:r1 - r0, :], in0=ot[:r1 - r0, :],
                                        in1=bo_sb[:r1 - r0, :], op=AL.add)
                nc.sync.dma_start(out=out[r0:r1, :], in_=ot[:r1 - r0, :])
    return nc


def kernel(**inputs):
    global _LAST_EXEC_NS
    import concourse.bass_utils as bass_utils
    bass_utils.upload_artifacts = lambda tmpdir: tmpdir
    from concourse.bass_utils import run_bass_kernel_spmd

    cfg = CFG
    per_core, tiles_w, ncalls = _prep(
        cfg, inputs["node_ids"], inputs["edge_src"], inputs["edge_dst"],
        inputs["graph_ids"], inputs["Ws"], inputs["bs"], inputs["bn_gamma"],
        inputs["bn_beta"], inputs["eps"], inputs["W_out"], inputs["b_out"],
        inputs["emb"])
    nc = _build(cfg, tiles_w, ncalls)
    nc.finalize()
    trace = bool(_PROFILE) and _install_profile_hook()
    res = run_bass_kernel_spmd(nc, per_core, core_ids=list(range(NCORES)),
                               trace=trace)
    _LAST_EXEC_NS = res.exec_time_ns
    return np.asarray(res.results[0]["out"], np.float32)

